# revision 58
# baseline (speedup 1.0000x reference)
# Multi-head causal attention (B=1, T=4096, D=1024, H=16) on 8 TRN2 NeuronCores.
#
# Sharding: tensor-parallel over heads. Core n computes head channels
# [128n, 128n+128) (= heads 2n, 2n+1), runs the full causal attention for its
# two heads, and produces a full-shape partial output
#   y_n = attn_out[:, ch_n] @ Wo[:, ch_n].T        (4096, 1024)
# The host sums the 8 partials (row-sharded Wo contraction) — no collectives.
#
# Device-side layout (per core):
#   xT   [128, 8, T]   x transposed + partition-tiled on the host so the DMA
#                      is contiguous; contraction (d_model) on partitions.
#   QT/KT [128, 2, T]  head channels on partitions (h0: 0-63, h1: 64-127), bf16
#   scoresT[j, i]      keys on partitions, queries on free dim; the softmax sum
#                      over keys rides the PV matmul via a ones-column appended
#                      to V (V' = [V | 1], M=65): psum row 64 = rowsum.
#   exp on ScalarE directly PSUM->SBUF (bf16 out) with 1/sqrt(dk) folded into
#   the activation scale; psum accumulation is always fp32.
#   Causal: only key tiles with j <= i are computed; diagonal 128x128 blocks
#   are masked by a 0/1 upper-triangular multiply after exp.
#
# Schedule: the ScalarE exp stream (144 ACTIVATEs of 128x1024, ~1.15us each)
# is the critical resource; everything is arranged to keep it saturated.
# Attention is a flat sequence of (key-tile-pair, head) steps per 512-query
# chunk; at step k the kernel emits score matmuls for step k+1, the exp for
# step k, and PV matmuls for step k-2 (the 2-step lag keeps psum-handoff
# waits at chunk boundaries off the in-order PE queue's critical path).
# PSUM banks: 2 rotating 2-bank score tiles (4), one 2-bank aux slot for
# projection/output-projection tiles (2), two 1-bank PV accumulators (2).
# The Q/K/V projection of chunk c+1 and the normalization+output projection
# tail of chunk c-1 are emitted interleaved into chunk c's attention steps;
# the tail's PE section (y matmuls) is gated to the last 45% of the chunk so
# the rowsum-reciprocal DMA roundtrip it depends on has completed.

import os
import sys

for _p in ("/opt/trn_rl_repo", "/root/.axon_site/_ro/trn_rl_repo"):
    if os.path.isdir(_p) and _p not in sys.path:
        sys.path.insert(0, _p)

import ml_dtypes
import numpy as np

def _ensure_axon_ntff_hook():
    """The agent image's antenv package lacks axon_hooks, which makes
    run_bass_kernel_spmd(trace=True) crash at import under axon. Provide the
    module and register the boot hook so NTFF profiling works."""
    import types

    try:
        import antenv.axon_hooks  # noqa: F401
        return
    except ImportError:
        pass
    try:
        import antenv
    except ImportError:
        return
    mod = types.ModuleType("antenv.axon_hooks")
    mod._hook = None
    mod.set_axon_ntff_profile_hook = lambda h: setattr(mod, "_hook", h)
    mod.get_axon_ntff_profile_hook = lambda: mod._hook
    sys.modules["antenv.axon_hooks"] = mod
    antenv.axon_hooks = mod
    try:
        from trn_agent_boot.trn_boot import _ntff_profile_via_ctypes

        so = "/opt/axon/libaxon_pjrt.so"
        if os.path.exists(so):
            mod._hook = _ntff_profile_via_ctypes(so)
    except Exception:
        pass


_ensure_axon_ntff_hook()

import concourse.bass as bass
import concourse.tile as tile
from concourse import bacc
from concourse import mybir
from concourse.bass_utils import run_bass_kernel_spmd

F32 = mybir.dt.float32
BF16 = mybir.dt.bfloat16
EXP = mybir.ActivationFunctionType.Exp
NPBF = ml_dtypes.bfloat16

D = 1024          # d_model
DK = 64           # head dim
CPC = 128         # channels per core (2 heads)
ICH = 512         # query-chunk size
IH = 512          # i-half width (matmul N / psum bank limit)
JT = 128          # key-tile size
NT = D // 128     # d_model tiles

_NC_CACHE = {}


def build(T):
    """Build the per-core Bass program for sequence length T."""
    nc = bacc.Bacc(None, target_bir_lowering=False, debug=False)
    ich = min(ICH, T)
    nch = T // ich

    xT_d = nc.dram_tensor(
        "xT", [T // ICH if T >= ICH else 1, 128, NT, min(ICH, T)], BF16,
        kind="ExternalInput",
    )
    wqT_d = nc.dram_tensor("wqT", [128, NT, CPC], BF16, kind="ExternalInput")
    wkT_d = nc.dram_tensor("wkT", [128, NT, CPC], BF16, kind="ExternalInput")
    wvT_d = nc.dram_tensor("wvT", [128, NT, CPC], BF16, kind="ExternalInput")
    woT_d = nc.dram_tensor("woT", [CPC, D], BF16, kind="ExternalInput")
    tri_d = nc.dram_tensor("tri", [JT, JT], BF16, kind="ExternalInput")
    ident_d = nc.dram_tensor("ident", [128, 128], BF16, kind="ExternalInput")
    y_d = nc.dram_tensor("y", [T, D], BF16, kind="ExternalOutput")

    with tile.TileContext(nc) as tc:
        with (
            tc.tile_pool(name="const", bufs=1) as const,
            tc.tile_pool(name="xtp", bufs=2) as xtp,
            tc.tile_pool(name="vtp", bufs=2) as vtp,
            tc.tile_pool(name="expp", bufs=6) as expp,
            tc.tile_pool(name="outp", bufs=2) as outp,
            tc.tile_pool(name="yp", bufs=4) as yp,
            tc.tile_pool(name="psp", bufs=2, space="PSUM") as psp,
            tc.tile_pool(name="pvp", bufs=1, space="PSUM") as pvp,
        ):
            xt_tiles = {}

            def prefetch_xt(c):
                xt_ch = xtp.tile([128, NT, ich], BF16, tag="xt", name="xt_ch")
                nc.sync.dma_start(out=xt_ch, in_=xT_d[c])
                xt_tiles[c] = xt_ch

            # ---- warm the ScalarE exp table while the first DMAs run ----
            warm = const.tile([128, 1], F32)
            nc.vector.memset(warm, 0.0)
            nc.scalar.activation(out=warm, in_=warm, func=EXP)

            # ---- HAM warm-up: ~3.5us of dummy matmuls (on garbage SBUF, to
            # a throwaway psum bank) so the PE clock-gate opens to 2.4 GHz
            # before the real projection matmuls arrive ----
            junk = const.tile([128, 128], BF16)
            nc.vector.memset(junk, 1.0)
            warm_ps = psp.tile([128, 2, IH], F32, tag="aux", bufs=1, name="warm_ps")
            for _ in range(30):
                nc.tensor.matmul(
                    out=warm_ps[:, 0, 0:128], lhsT=junk, rhs=junk,
                    start=True, stop=True,
                )

            # ---- constants / persistent state; DMA trigger order matters:
            # the sync queue issues one ~0.6us trigger at a time, so the
            # operands of the first projection matmuls go first ----
            wq_sb = const.tile([128, NT, 128], BF16)
            wk_sb = const.tile([128, NT, 128], BF16)
            wv_sb = const.tile([128, NT, 128], BF16)
            wo_sb = const.tile([128, D], BF16)
            tri_sb = const.tile([JT, JT], BF16)
            id_sb = const.tile([128, 128], BF16)

            xt_ch0 = xtp.tile([128, NT, ich], BF16, tag="xt", name="xt_ch")
            xt_tiles[0] = xt_ch0
            nc.sync.dma_start(out=wq_sb, in_=wqT_d[:, :, :])
            nc.sync.dma_start(out=xt_ch0[:, 0:2, :], in_=xT_d[0, :, 0:2, :])
            nc.sync.dma_start(out=xt_ch0[:, 2:NT, :], in_=xT_d[0, :, 2:NT, :])
            nc.sync.dma_start(out=wk_sb, in_=wkT_d[:, :, :])
            nc.sync.dma_start(out=wv_sb, in_=wvT_d[:, :, :])
            if nch > 1:
                prefetch_xt(1)
            nc.sync.dma_start(out=wo_sb, in_=woT_d[:, :])
            nc.sync.dma_start(out=tri_sb, in_=tri_d[:, :])
            nc.sync.dma_start(out=id_sb, in_=ident_d[:, :])

            ones_col = const.tile([1, DK], F32)
            nc.vector.memset(ones_col, 1.0)

            qt_sb = const.tile([128, 2, T], BF16)  # [:,0,:]=QT, [:,1,:]=KT
            # V' = [V_h | 1] per head: [j, jt, 2*65]
            vp_sb = const.tile([128, T // JT, 2 * (DK + 1)], BF16)
            ones_view = vp_sb.rearrange("p t (h c) -> p t h c", h=2)[
                :, :, :, DK : DK + 1
            ]
            nc.vector.memset(ones_view, 1.0)

            def gen_proj(c):
                """Generator emitting the Q/K/V projection for chunk c,
                one PE/DVE op per yield (the x chunk was prefetched).
                NOTE: must be fully emitted before chunk c's attention —
                the vp_sb copy's rearranged write is not reliably ordered
                against the PV matmuls by the dependency tracker."""
                i0 = c * ich
                xt_ch = xt_tiles.pop(c)
                # interleave the Q and K accumulation chains: consecutive
                # matmuls into the SAME psum bank pace at ~429ns (the bank
                # read-modify-write can't overlap the next fill) while
                # bank-alternating matmuls pace at ~215ns
                # split each K=128xM=128 projection matmul into two
                # CONCURRENT M=64 col-tiles (disjoint output partition
                # halves): full-array stationary operands pace at ~429ns
                # while a col-tiled pair shares one ~320ns window
                qk_ps = psp.tile([128, 2, ich], F32, tag="aux", bufs=1, name="qk_ps")
                for qk, w_sb in ((0, wq_sb), (1, wk_sb)):
                    for t in range(NT):
                        for m in (slice(0, 64), slice(64, 128)):
                            nc.tensor.matmul(
                                out=qk_ps[m, qk, :],
                                lhsT=w_sb[:, t, m],
                                rhs=xt_ch[:, t, :],
                                start=(t == 0),
                                stop=(t == NT - 1),
                            )
                        yield
                nc.vector.tensor_copy(out=qt_sb[:, :, i0 : i0 + ich], in_=qk_ps)
                yield
                vt_ps = psp.tile([128, 2, ich], F32, tag="aux", bufs=1, name="vt_ps")
                for t in range(NT):
                    for m in (slice(0, 64), slice(64, 128)):
                        nc.tensor.matmul(
                            out=vt_ps[m, 0, :],
                            lhsT=wv_sb[:, t, m],
                            rhs=xt_ch[:, t, :],
                            start=(t == 0),
                            stop=(t == NT - 1),
                        )
                    yield
                vt_sb = vtp.tile([128, ich], BF16, tag="vt", name="vt_sb")
                nc.vector.tensor_copy(out=vt_sb, in_=vt_ps[:, 0, :])
                yield
                vn_ps = psp.tile(
                    [128, ich // 128, 128], BF16, tag="aux", bufs=1, name="vn_ps"
                )
                for sdx in range(ich // 128):
                    nc.tensor.transpose(
                        out=vn_ps[:, sdx, :],
                        in_=vt_sb[:, sdx * 128 : (sdx + 1) * 128],
                        identity=id_sb,
                    )
                    yield
                jt0 = i0 // JT
                nc.vector.tensor_copy(
                    out=vp_sb.rearrange("p t (h c) -> p t h c", h=2)[
                        :, jt0 : jt0 + ich // 128, :, 0:DK
                    ],
                    in_=vn_ps.rearrange("p s (h c) -> p s h c", h=2),
                )
                yield

            def gen_tail_head(c, pv, out_slot, last=False):
                """Rowsum + raw-output extraction out of the pv psum banks —
                emitted first thing in the next chunk so the banks free up
                for its PV accumulation. After the final chunk the scalar
                engine is idle, so the rowsum copies run there, in parallel
                with the vector-queue outt copies."""
                rs_sb = outp.tile([1, 2, ich], F32, tag="rs", name="rs_sb")
                for h in range(2):
                    if last:
                        nc.scalar.copy(
                            out=rs_sb[0:1, h, :], in_=pv[h][DK : DK + 1, :]
                        )
                    else:
                        nc.vector.tensor_copy(
                            out=rs_sb[0:1, h, :], in_=pv[h][DK : DK + 1, :]
                        )
                outt = outp.tile([128, ich], BF16, tag="outt", name="outt")
                for h in range(2):
                    nc.vector.tensor_copy(
                        out=outt[h * DK : (h + 1) * DK, :], in_=pv[h][0:DK, :]
                    )
                out_slot["rs"] = rs_sb
                out_slot["outt"] = outt
                yield
                # rowsum broadcast to the 64 head channels via two K=1
                # ones-matmuls (cheaper and far lower latency than a
                # scratch-DRAM roundtrip); needs only rs_sb, so the PE
                # reaches it with its input ready
                bc_ps = psp.tile(
                    [128, 2, IH], F32, tag="aux", bufs=1, name="bc_ps"
                )
                for h in range(2):
                    nc.tensor.matmul(
                        out=bc_ps[0:DK, h, :],
                        lhsT=ones_col,
                        rhs=rs_sb[0:1, h, :],
                        start=True,
                        stop=True,
                    )
                out_slot["bc_ps"] = bc_ps
                yield

            def gen_tail_bc(c, out_slot):
                """DVE-only normalization: fast approximate reciprocal
                (18 bits — plenty for a softmax denominator; rowsums are
                strictly positive) of the broadcast rowsums, then scale the
                raw attention output. No PE ops — cannot stall the PE
                queue."""
                outt, bc_ps = out_slot["outt"], out_slot["bc_ps"]
                # the custom-DVE approx reciprocal requires matching
                # partition bases, so invert at base 0 then shift-copy
                bct = outp.tile([DK, 2, ich], F32, tag="bct", name="bct")
                for h in range(2):
                    nc.vector.reciprocal_approx_fast(
                        out=bct[:, h, :], in_=bc_ps[0:DK, h, :]
                    )
                yield
                bc = outp.tile([128, ich], F32, tag="bc", name="bc")
                for h in range(2):
                    nc.vector.tensor_copy(
                        out=bc[h * DK : (h + 1) * DK, :], in_=bct[:, h, :]
                    )
                    yield
                # per-half muls so the first y matmuls (reading the first
                # 256 queries) can start before the whole tile is scaled
                for q in (slice(0, ich // 2), slice(ich // 2, ich)):
                    nc.vector.tensor_mul(
                        outt[:, q], outt[:, q], bc[:, q]
                    )
                    yield

            def gen_tail_y(c, out_slot, last=False):
                """Output projection, gated late so the y matmuls reach the
                in-order PE queue only after the normalization chain is
                done. The last chunk's y tiles use the score rotation (free
                by then) so its output projection pipelines 2-deep."""
                i0 = c * ich
                outt = out_slot["outt"]
                for sidx in range(ich // 128):
                    y_ps = psp.tile(
                        [128, 2, IH], F32,
                        tag="sc" if last else "aux",
                        bufs=2 if last else 1,
                        name="y_ps",
                    )
                    for e in range(2):
                        for m in (slice(0, 64), slice(64, 128)):
                            nc.tensor.matmul(
                                out=y_ps[m, e, :],
                                lhsT=outt[
                                    :,
                                    sidx * 128 + m.start : sidx * 128 + m.stop,
                                ],
                                rhs=wo_sb[:, e * IH : (e + 1) * IH],
                                start=True,
                                stop=True,
                            )
                        yield
                    y_sb = yp.tile([128, D], BF16, tag="y", name="y_sb")
                    nc.vector.tensor_copy(
                        out=y_sb, in_=y_ps.rearrange("p a b -> p (a b)")
                    )
                    yield
                    r0 = i0 + sidx * 128
                    nc.sync.dma_start(out=y_d[r0 : r0 + 128, :], in_=y_sb)
                    yield

            def emit_chunk(c, pending):
                """Attention steps for chunk c, draining `pending` generator
                entries [min_frac, gen] into the step slack."""
                i0 = c * ich
                njt = (i0 + ich) // JT
                pv = [
                    pvp.tile([128, ich], F32, tag="pv0", name="pv0"),
                    pvp.tile([128, ich], F32, tag="pv1", name="pv1"),
                ]
                # one step per 128-key tile; both heads' K=64 score matmuls
                # are row-tiled (tile_position auto-derives from the lhsT/out
                # base partitions: h0 rows 0-63, h1 rows 64-127) and execute
                # CONCURRENTLY in the PE array — one N=512 window for both
                nsteps = njt
                sc_tiles = {}
                ex_tiles = {}

                def emit_sc(k):
                    sc = psp.tile([128, 2, IH], F32, tag="sc", name="sc")
                    sc_tiles[k] = sc
                    for h in range(2):
                        hp = slice(h * DK, (h + 1) * DK)
                        nc.tensor.matmul(
                            out=sc[:, h, :],
                            lhsT=qt_sb[hp, 1, k * JT : (k + 1) * JT],
                            rhs=qt_sb[hp, 0, i0 : i0 + ich],
                            start=True,
                            stop=True,
                        )

                def emit_exp(k):
                    sc = sc_tiles.pop(k)
                    ex = expp.tile([128, 2, IH], BF16, tag="ex", name="ex")
                    ex_tiles[k] = ex
                    nc.scalar.activation(
                        out=ex, in_=sc, func=EXP, scale=1.0 / np.sqrt(DK)
                    )
                    off = k * JT - i0
                    if off >= 0:  # diagonal tile: causal mask
                        if off > 0:
                            nc.vector.memset(ex[:, :, 0:off], 0.0)
                        for h in range(2):
                            nc.vector.tensor_mul(
                                ex[:, h, off : off + JT],
                                ex[:, h, off : off + JT],
                                tri_sb,
                            )

                def emit_pv(k):
                    ex = ex_tiles.pop(k)
                    for h in range(2):
                        nc.tensor.matmul(
                            out=pv[h][0 : DK + 1, :],
                            lhsT=vp_sb[:, k, h * (DK + 1) : (h + 1) * (DK + 1)],
                            rhs=ex[:, h, :],
                            start=(k == 0),
                            stop=(k == njt - 1),
                        )

                def drain(budget, frac):
                    budget += 2 if frac > 0.8 else 0
                    while budget > 0:
                        eligible = [
                            e for e in pending
                            if e[2] < c or (e[2] == c and e[0] <= frac)
                        ]
                        if not eligible:
                            return
                        progressed = False
                        for e in eligible:
                            if budget <= 0:
                                return
                            try:
                                next(e[1])
                                budget -= 1
                                progressed = True
                            except StopIteration:
                                pending.remove(e)
                        if not progressed:
                            return

                total_ops = sum(
                    20 if e[0] == 0.0 else 14
                    for e in pending if e[2] <= c
                )
                emit_sc(0)
                # the previous chunk's 2 lagging PV pairs are emitted here,
                # AFTER this chunk's first score window, so the next exp is
                # never queued behind them at the boundary
                for fn in prev_trailing:
                    fn()
                for k in range(nsteps):
                    if k + 1 < nsteps:
                        emit_sc(k + 1)
                    emit_exp(k)
                    if k - 2 >= 0:
                        emit_pv(k - 2)
                    drain(
                        (total_ops + nsteps - 1 - k) // nsteps + 1,
                        (k + 1) / nsteps,
                    )
                trailing = [
                    (lambda k=k: emit_pv(k))
                    for k in (nsteps - 2, nsteps - 1)
                ]
                return pv, trailing

            # ---- main schedule ----
            for _ in gen_proj(0):
                pass
            pending = []
            prev_trailing = []
            for c in range(nch):
                if c + 2 < nch:
                    prefetch_xt(c + 2)
                if c + 1 < nch:
                    pending.append([0.0, gen_proj(c + 1), c])
                pv, prev_trailing = emit_chunk(c, pending)
                out_slot = {}
                pending.append(
                    [0.0, gen_tail_head(c, pv, out_slot, last=(c == nch - 1)),
                     c + 1]
                )
                pending.append([0.20, gen_tail_bc(c, out_slot), c + 1])
                # early chunks are PE-oversubscribed (attention + next
                # chunk's projection exceed their few steps of scalar
                # time), so their output projections run two chunks later
                # where the bigger chunks have PE slack
                ydef = c + 2 if c < nch - 3 else c + 1
                pending.append(
                    [0.55, gen_tail_y(c, out_slot, last=(c == nch - 1)), ydef]
                )
            for fn in prev_trailing:
                fn()
            for e in pending:
                for _ in e[1]:
                    pass
    nc.compile()
    return nc


def get_nc(T):
    if T not in _NC_CACHE:
        _NC_CACHE[T] = build(T)
    return _NC_CACHE[T]


TRI = np.triu(np.ones((JT, JT))).astype(NPBF)  # 1 where key j <= query i
IDENT = np.eye(128).astype(NPBF)

LAST_RESULTS = None  # BassKernelResults of the last run (for profiling)


def _tile_dk(w):
    """[D, C] -> [128, D//128, C] partition-tiled so the device DMA is
    contiguous: out[p, t, c] = w[t*128 + p, c]."""
    Dd, C = w.shape
    return np.ascontiguousarray(
        w.reshape(Dd // 128, 128, C).transpose(1, 0, 2)
    )


def make_in_maps(x, Wq, Wk, Wv, Wo, n_cores=8):
    """x: (T, D) fp32. Returns per-core input maps (bf16 operands)."""
    T = x.shape[0]
    ich = min(ICH, T)
    # [nch, 128, NT, ich]: chunk-contiguous so each chunk is one linear DMA
    xT = np.ascontiguousarray(
        x.T.astype(NPBF)
        .reshape(NT, 128, T // ich, ich)
        .transpose(2, 1, 0, 3)
    )
    maps = []
    for n in range(n_cores):
        sl = slice(CPC * n, CPC * (n + 1))
        maps.append(
            {
                "xT": xT,
                "wqT": _tile_dk(Wq[sl, :].T.astype(NPBF)),
                "wkT": _tile_dk(Wk[sl, :].T.astype(NPBF)),
                "wvT": _tile_dk(Wv[sl, :].T.astype(NPBF)),
                "woT": np.ascontiguousarray(Wo[:, sl].T).astype(NPBF),
                "tri": TRI,
                "ident": IDENT,
            }
        )
    return maps


def run(x, Wq, Wk, Wv, Wo, T=None, n_cores=8, trace=False):
    global LAST_RESULTS
    T = T if T is not None else x.shape[0]
    nc = get_nc(T)
    in_maps = make_in_maps(x, Wq, Wk, Wv, Wo, n_cores)
    res = run_bass_kernel_spmd(
        nc, in_maps, core_ids=list(range(n_cores)), trace=trace
    )
    LAST_RESULTS = res
    y = np.zeros((T, D), dtype=np.float64)
    for r in res.results:
        y += r["y"].astype(np.float64)
    return y.astype(np.float32)


def kernel(x, Wq, Wk, Wv, Wo):
    x = np.asarray(x, dtype=np.float32)
    B, T, _ = x.shape
    trace = bool(os.environ.get("MHA_TRACE"))
    y = run(
        np.ascontiguousarray(x.reshape(T, D)),
        np.asarray(Wq, np.float32),
        np.asarray(Wk, np.float32),
        np.asarray(Wv, np.float32),
        np.asarray(Wo, np.float32),
        T=T,
        trace=trace,
    )
    if trace and LAST_RESULTS is not None and LAST_RESULTS.exec_time_ns:
        print(f"HW exec time: {LAST_RESULTS.exec_time_ns} ns")
    return y.reshape(B, T, D)


# revision 60
# speedup vs baseline: 1.0419x; 1.0419x over previous
# Multi-head causal attention (B=1, T=4096, D=1024, H=16) on 8 TRN2 NeuronCores.
#
# Sharding: tensor-parallel over heads. Core n computes head channels
# [128n, 128n+128) (= heads 2n, 2n+1), runs the full causal attention for its
# two heads, and produces a full-shape partial output
#   y_n = attn_out[:, ch_n] @ Wo[:, ch_n].T        (4096, 1024)
# The host sums the 8 partials (row-sharded Wo contraction) — no collectives.
#
# Device-side layout (per core):
#   xT   [128, 8, T]   x transposed + partition-tiled on the host so the DMA
#                      is contiguous; contraction (d_model) on partitions.
#   QT/KT [128, 2, T]  head channels on partitions (h0: 0-63, h1: 64-127), bf16
#   scoresT[j, i]      keys on partitions, queries on free dim; the softmax sum
#                      over keys rides the PV matmul via a ones-column appended
#                      to V (V' = [V | 1], M=65): psum row 64 = rowsum.
#   exp on ScalarE directly PSUM->SBUF (bf16 out) with 1/sqrt(dk) folded into
#   the activation scale; psum accumulation is always fp32.
#   Causal: only key tiles with j <= i are computed; diagonal 128x128 blocks
#   are masked by a 0/1 upper-triangular multiply after exp.
#
# Schedule: the ScalarE exp stream (144 ACTIVATEs of 128x1024, ~1.15us each)
# is the critical resource; everything is arranged to keep it saturated.
# Attention is a flat sequence of (key-tile-pair, head) steps per 512-query
# chunk; at step k the kernel emits score matmuls for step k+1, the exp for
# step k, and PV matmuls for step k-2 (the 2-step lag keeps psum-handoff
# waits at chunk boundaries off the in-order PE queue's critical path).
# PSUM banks: 2 rotating 2-bank score tiles (4), one 2-bank aux slot for
# projection/output-projection tiles (2), two 1-bank PV accumulators (2).
# The Q/K/V projection of chunk c+1 and the normalization+output projection
# tail of chunk c-1 are emitted interleaved into chunk c's attention steps;
# the tail's PE section (y matmuls) is gated to the last 45% of the chunk so
# the rowsum-reciprocal DMA roundtrip it depends on has completed.

import os
import sys

for _p in ("/opt/trn_rl_repo", "/root/.axon_site/_ro/trn_rl_repo"):
    if os.path.isdir(_p) and _p not in sys.path:
        sys.path.insert(0, _p)

import ml_dtypes
import numpy as np

def _ensure_axon_ntff_hook():
    """The agent image's antenv package lacks axon_hooks, which makes
    run_bass_kernel_spmd(trace=True) crash at import under axon. Provide the
    module and register the boot hook so NTFF profiling works."""
    import types

    try:
        import antenv.axon_hooks  # noqa: F401
        return
    except ImportError:
        pass
    try:
        import antenv
    except ImportError:
        return
    mod = types.ModuleType("antenv.axon_hooks")
    mod._hook = None
    mod.set_axon_ntff_profile_hook = lambda h: setattr(mod, "_hook", h)
    mod.get_axon_ntff_profile_hook = lambda: mod._hook
    sys.modules["antenv.axon_hooks"] = mod
    antenv.axon_hooks = mod
    try:
        from trn_agent_boot.trn_boot import _ntff_profile_via_ctypes

        so = "/opt/axon/libaxon_pjrt.so"
        if os.path.exists(so):
            mod._hook = _ntff_profile_via_ctypes(so)
    except Exception:
        pass


_ensure_axon_ntff_hook()

import concourse.bass as bass
import concourse.tile as tile
from concourse import bacc
from concourse import mybir
from concourse.bass_utils import run_bass_kernel_spmd

F32 = mybir.dt.float32
BF16 = mybir.dt.bfloat16
EXP = mybir.ActivationFunctionType.Exp
NPBF = ml_dtypes.bfloat16

D = 1024          # d_model
DK = 64           # head dim
CPC = 128         # channels per core (2 heads)
ICH = 512         # query-chunk size
IH = 512          # i-half width (matmul N / psum bank limit)
JT = 128          # key-tile size
NT = D // 128     # d_model tiles

_NC_CACHE = {}


def build(T):
    """Build the per-core Bass program for sequence length T."""
    nc = bacc.Bacc(None, target_bir_lowering=False, debug=False)
    ich = min(ICH, T)
    nch = T // ich

    xT_d = nc.dram_tensor(
        "xT", [T // ICH if T >= ICH else 1, 128, NT, min(ICH, T)], BF16,
        kind="ExternalInput",
    )
    wqT_d = nc.dram_tensor("wqT", [128, NT, CPC], BF16, kind="ExternalInput")
    wkT_d = nc.dram_tensor("wkT", [128, NT, CPC], BF16, kind="ExternalInput")
    wvT_d = nc.dram_tensor("wvT", [128, NT, CPC], BF16, kind="ExternalInput")
    woT_d = nc.dram_tensor("woT", [CPC, D], BF16, kind="ExternalInput")
    tri_d = nc.dram_tensor("tri", [JT, JT], BF16, kind="ExternalInput")
    ident_d = nc.dram_tensor("ident", [128, 128], BF16, kind="ExternalInput")
    y_d = nc.dram_tensor("y", [T, D], BF16, kind="ExternalOutput")

    with tile.TileContext(nc) as tc:
        with (
            tc.tile_pool(name="const", bufs=1) as const,
            tc.tile_pool(name="xtp", bufs=2) as xtp,
            tc.tile_pool(name="vtp", bufs=2) as vtp,
            tc.tile_pool(name="expp", bufs=6) as expp,
            tc.tile_pool(name="outp", bufs=2) as outp,
            tc.tile_pool(name="yp", bufs=4) as yp,
            tc.tile_pool(name="psp", bufs=2, space="PSUM") as psp,
            tc.tile_pool(name="pvp", bufs=1, space="PSUM") as pvp,
        ):
            xt_tiles = {}

            def prefetch_xt(c):
                xt_ch = xtp.tile([128, NT, ich], BF16, tag="xt", name="xt_ch")
                nc.sync.dma_start(out=xt_ch, in_=xT_d[c])
                xt_tiles[c] = xt_ch

            # ---- warm the ScalarE exp table while the first DMAs run ----
            warm = const.tile([128, 1], F32)
            nc.vector.memset(warm, 0.0)
            nc.scalar.activation(out=warm, in_=warm, func=EXP)

            # ---- HAM warm-up: ~3.5us of dummy matmuls (on garbage SBUF, to
            # a throwaway psum bank) so the PE clock-gate opens to 2.4 GHz
            # before the real projection matmuls arrive ----
            junk = const.tile([128, 128], BF16)
            nc.vector.memset(junk, 1.0)
            warm_ps = psp.tile([128, 2, IH], F32, tag="aux", bufs=1, name="warm_ps")
            for _ in range(30):
                nc.tensor.matmul(
                    out=warm_ps[:, 0, 0:128], lhsT=junk, rhs=junk,
                    start=True, stop=True,
                )

            # ---- constants / persistent state; DMA trigger order matters:
            # the sync queue issues one ~0.6us trigger at a time, so the
            # operands of the first projection matmuls go first ----
            wq_sb = const.tile([128, NT, 128], BF16)
            wk_sb = const.tile([128, NT, 128], BF16)
            wv_sb = const.tile([128, NT, 128], BF16)
            wo_sb = const.tile([128, D], BF16)
            tri_sb = const.tile([JT, JT], BF16)
            id_sb = const.tile([128, 128], BF16)

            xt_ch0 = xtp.tile([128, NT, ich], BF16, tag="xt", name="xt_ch")
            xt_tiles[0] = xt_ch0
            nc.sync.dma_start(out=wq_sb, in_=wqT_d[:, :, :])
            nc.sync.dma_start(out=xt_ch0[:, 0:2, :], in_=xT_d[0, :, 0:2, :])
            nc.sync.dma_start(out=xt_ch0[:, 2:NT, :], in_=xT_d[0, :, 2:NT, :])
            nc.sync.dma_start(out=wk_sb, in_=wkT_d[:, :, :])
            nc.sync.dma_start(out=wv_sb, in_=wvT_d[:, :, :])
            if nch > 1:
                prefetch_xt(1)
            nc.sync.dma_start(out=wo_sb, in_=woT_d[:, :])
            nc.sync.dma_start(out=tri_sb, in_=tri_d[:, :])
            nc.sync.dma_start(out=id_sb, in_=ident_d[:, :])

            ones_col = const.tile([1, DK], F32)
            nc.vector.memset(ones_col, 1.0)

            qt_sb = const.tile([128, 2, T], BF16)  # [:,0,:]=QT, [:,1,:]=KT
            # V' = [V_h | 1] per head: [j, jt, 2*65]
            vp_sb = const.tile([128, T // JT, 2 * (DK + 1)], BF16)
            ones_view = vp_sb.rearrange("p t (h c) -> p t h c", h=2)[
                :, :, :, DK : DK + 1
            ]
            nc.vector.memset(ones_view, 1.0)

            def gen_proj(c):
                """Generator emitting the Q/K/V projection for chunk c,
                one PE/DVE op per yield (the x chunk was prefetched).
                NOTE: must be fully emitted before chunk c's attention —
                the vp_sb copy's rearranged write is not reliably ordered
                against the PV matmuls by the dependency tracker."""
                i0 = c * ich
                xt_ch = xt_tiles.pop(c)
                # interleave the Q and K accumulation chains: consecutive
                # matmuls into the SAME psum bank pace at ~429ns (the bank
                # read-modify-write can't overlap the next fill) while
                # bank-alternating matmuls pace at ~215ns
                qk_ps = psp.tile([128, 2, ich], F32, tag="aux", bufs=1, name="qk_ps")
                for qk, w_sb in ((0, wq_sb), (1, wk_sb)):
                    for t in range(NT):
                        nc.tensor.matmul(
                            out=qk_ps[:, qk, :],
                            lhsT=w_sb[:, t, :],
                            rhs=xt_ch[:, t, :],
                            start=(t == 0),
                            stop=(t == NT - 1),
                        )
                        yield
                nc.vector.tensor_copy(out=qt_sb[:, :, i0 : i0 + ich], in_=qk_ps)
                yield
                vt_ps = psp.tile([128, 2, ich], F32, tag="aux", bufs=1, name="vt_ps")
                for t in range(NT):
                    nc.tensor.matmul(
                        out=vt_ps[:, 0, :],
                        lhsT=wv_sb[:, t, :],
                        rhs=xt_ch[:, t, :],
                        start=(t == 0),
                        stop=(t == NT - 1),
                    )
                    yield
                vt_sb = vtp.tile([128, ich], BF16, tag="vt", name="vt_sb")
                nc.vector.tensor_copy(out=vt_sb, in_=vt_ps[:, 0, :])
                yield
                vn_ps = psp.tile(
                    [128, ich // 128, 128], BF16, tag="aux", bufs=1, name="vn_ps"
                )
                for sdx in range(ich // 128):
                    nc.tensor.transpose(
                        out=vn_ps[:, sdx, :],
                        in_=vt_sb[:, sdx * 128 : (sdx + 1) * 128],
                        identity=id_sb,
                    )
                    yield
                jt0 = i0 // JT
                nc.vector.tensor_copy(
                    out=vp_sb.rearrange("p t (h c) -> p t h c", h=2)[
                        :, jt0 : jt0 + ich // 128, :, 0:DK
                    ],
                    in_=vn_ps.rearrange("p s (h c) -> p s h c", h=2),
                )
                yield

            def gen_tail_head(c, pv, out_slot, last=False):
                """Rowsum + raw-output extraction out of the pv psum banks —
                emitted first thing in the next chunk so the banks free up
                for its PV accumulation. After the final chunk the scalar
                engine is idle, so the rowsum copies run there, in parallel
                with the vector-queue outt copies."""
                rs_sb = outp.tile([1, 2, ich], F32, tag="rs", name="rs_sb")
                for h in range(2):
                    if last:
                        nc.scalar.copy(
                            out=rs_sb[0:1, h, :], in_=pv[h][DK : DK + 1, :]
                        )
                    else:
                        nc.vector.tensor_copy(
                            out=rs_sb[0:1, h, :], in_=pv[h][DK : DK + 1, :]
                        )
                outt = outp.tile([128, ich], BF16, tag="outt", name="outt")
                for h in range(2):
                    nc.vector.tensor_copy(
                        out=outt[h * DK : (h + 1) * DK, :], in_=pv[h][0:DK, :]
                    )
                out_slot["rs"] = rs_sb
                out_slot["outt"] = outt
                yield
                # rowsum broadcast to the 64 head channels via two K=1
                # ones-matmuls (cheaper and far lower latency than a
                # scratch-DRAM roundtrip); needs only rs_sb, so the PE
                # reaches it with its input ready
                bc_ps = psp.tile(
                    [128, 2, IH], F32, tag="aux", bufs=1, name="bc_ps"
                )
                for h in range(2):
                    nc.tensor.matmul(
                        out=bc_ps[0:DK, h, :],
                        lhsT=ones_col,
                        rhs=rs_sb[0:1, h, :],
                        start=True,
                        stop=True,
                    )
                out_slot["bc_ps"] = bc_ps
                yield

            def gen_tail_bc(c, out_slot):
                """DVE-only normalization: fast approximate reciprocal
                (18 bits — plenty for a softmax denominator; rowsums are
                strictly positive) of the broadcast rowsums, then scale the
                raw attention output. No PE ops — cannot stall the PE
                queue."""
                outt, bc_ps = out_slot["outt"], out_slot["bc_ps"]
                # the custom-DVE approx reciprocal requires matching
                # partition bases, so invert at base 0 then shift-copy
                bct = outp.tile([DK, 2, ich], F32, tag="bct", name="bct")
                for h in range(2):
                    nc.vector.reciprocal_approx_fast(
                        out=bct[:, h, :], in_=bc_ps[0:DK, h, :]
                    )
                yield
                bc = outp.tile([128, ich], F32, tag="bc", name="bc")
                for h in range(2):
                    nc.vector.tensor_copy(
                        out=bc[h * DK : (h + 1) * DK, :], in_=bct[:, h, :]
                    )
                    yield
                # per-half muls so the first y matmuls (reading the first
                # 256 queries) can start before the whole tile is scaled
                for q in (slice(0, ich // 2), slice(ich // 2, ich)):
                    nc.vector.tensor_mul(
                        outt[:, q], outt[:, q], bc[:, q]
                    )
                    yield

            def gen_tail_y(c, out_slot, last=False):
                """Output projection, gated late so the y matmuls reach the
                in-order PE queue only after the normalization chain is
                done. The last chunk's y tiles use the score rotation (free
                by then) so its output projection pipelines 2-deep."""
                i0 = c * ich
                outt = out_slot["outt"]
                for sidx in range(ich // 128):
                    y_ps = psp.tile(
                        [128, 2, IH], F32,
                        tag="sc" if last else "aux",
                        bufs=2 if last else 1,
                        name="y_ps",
                    )
                    for e in range(2):
                        nc.tensor.matmul(
                            out=y_ps[:, e, :],
                            lhsT=outt[:, sidx * 128 : (sidx + 1) * 128],
                            rhs=wo_sb[:, e * IH : (e + 1) * IH],
                            start=True,
                            stop=True,
                        )
                        yield
                    y_sb = yp.tile([128, D], BF16, tag="y", name="y_sb")
                    nc.vector.tensor_copy(
                        out=y_sb, in_=y_ps.rearrange("p a b -> p (a b)")
                    )
                    yield
                    r0 = i0 + sidx * 128
                    nc.sync.dma_start(out=y_d[r0 : r0 + 128, :], in_=y_sb)
                    yield

            def emit_chunk(c, pending):
                """Attention steps for chunk c, draining `pending` generator
                entries [min_frac, gen] into the step slack."""
                i0 = c * ich
                njt = (i0 + ich) // JT
                pv = [
                    pvp.tile([128, ich], F32, tag="pv0", name="pv0"),
                    pvp.tile([128, ich], F32, tag="pv1", name="pv1"),
                ]
                # one step per 128-key tile; both heads' K=64 score matmuls
                # are row-tiled (tile_position auto-derives from the lhsT/out
                # base partitions: h0 rows 0-63, h1 rows 64-127) and execute
                # CONCURRENTLY in the PE array — one N=512 window for both
                nsteps = njt
                sc_tiles = {}
                ex_tiles = {}

                def emit_sc(k):
                    sc = psp.tile([128, 2, IH], F32, tag="sc", name="sc")
                    sc_tiles[k] = sc
                    for h in range(2):
                        hp = slice(h * DK, (h + 1) * DK)
                        nc.tensor.matmul(
                            out=sc[:, h, :],
                            lhsT=qt_sb[hp, 1, k * JT : (k + 1) * JT],
                            rhs=qt_sb[hp, 0, i0 : i0 + ich],
                            start=True,
                            stop=True,
                        )

                def emit_exp(k):
                    sc = sc_tiles.pop(k)
                    ex = expp.tile([128, 2, IH], BF16, tag="ex", name="ex")
                    ex_tiles[k] = ex
                    nc.scalar.activation(
                        out=ex, in_=sc, func=EXP, scale=1.0 / np.sqrt(DK)
                    )
                    off = k * JT - i0
                    if off >= 0:  # diagonal tile: causal mask (on GpSimd —
                        # it is idle, and this keeps the masks off the
                        # vector queue where drained copies could delay them)
                        if off > 0:
                            nc.gpsimd.memset(ex[:, :, 0:off], 0.0)
                        for h in range(2):
                            nc.gpsimd.tensor_mul(
                                ex[:, h, off : off + JT],
                                ex[:, h, off : off + JT],
                                tri_sb,
                            )

                def emit_pv(k):
                    ex = ex_tiles.pop(k)
                    for h in range(2):
                        nc.tensor.matmul(
                            out=pv[h][0 : DK + 1, :],
                            lhsT=vp_sb[:, k, h * (DK + 1) : (h + 1) * (DK + 1)],
                            rhs=ex[:, h, :],
                            start=(k == 0),
                            stop=(k == njt - 1),
                        )

                def drain(budget, frac):
                    budget += 2 if frac > 0.8 else 0
                    while budget > 0:
                        eligible = [
                            e for e in pending
                            if e[2] < c or (e[2] == c and e[0] <= frac)
                        ]
                        if not eligible:
                            return
                        progressed = False
                        for e in eligible:
                            if budget <= 0:
                                return
                            try:
                                next(e[1])
                                budget -= 1
                                progressed = True
                            except StopIteration:
                                pending.remove(e)
                        if not progressed:
                            return

                total_ops = sum(
                    20 if e[0] == 0.0 else 14
                    for e in pending if e[2] <= c
                )
                emit_sc(0)
                # the previous chunk's 2 lagging PV pairs are emitted here,
                # AFTER this chunk's first score window, so the next exp is
                # never queued behind them at the boundary
                for fn in prev_trailing:
                    fn()
                for k in range(nsteps):
                    if k + 1 < nsteps:
                        emit_sc(k + 1)
                    emit_exp(k)
                    if k - 2 >= 0:
                        emit_pv(k - 2)
                    drain(
                        (total_ops + nsteps - 1 - k) // nsteps + 1,
                        (k + 1) / nsteps,
                    )
                trailing = [
                    (lambda k=k: emit_pv(k))
                    for k in (nsteps - 2, nsteps - 1)
                ]
                return pv, trailing

            # ---- main schedule ----
            for _ in gen_proj(0):
                pass
            pending = []
            prev_trailing = []
            for c in range(nch):
                if c + 2 < nch:
                    prefetch_xt(c + 2)
                if c + 1 < nch:
                    pending.append([0.0, gen_proj(c + 1), c])
                pv, prev_trailing = emit_chunk(c, pending)
                out_slot = {}
                pending.append(
                    [0.0, gen_tail_head(c, pv, out_slot, last=(c == nch - 1)),
                     c + 1]
                )
                pending.append([0.20, gen_tail_bc(c, out_slot), c + 1])
                # early chunks are PE-oversubscribed (attention + next
                # chunk's projection exceed their few steps of scalar
                # time), so their output projections run two chunks later
                # where the bigger chunks have PE slack
                ydef = c + 2 if c < nch - 3 else c + 1
                pending.append(
                    [0.55, gen_tail_y(c, out_slot, last=(c == nch - 1)), ydef]
                )
            for fn in prev_trailing:
                fn()
            for e in pending:
                for _ in e[1]:
                    pass
    nc.compile()
    return nc


def get_nc(T):
    if T not in _NC_CACHE:
        _NC_CACHE[T] = build(T)
    return _NC_CACHE[T]


TRI = np.triu(np.ones((JT, JT))).astype(NPBF)  # 1 where key j <= query i
IDENT = np.eye(128).astype(NPBF)

LAST_RESULTS = None  # BassKernelResults of the last run (for profiling)


def _tile_dk(w):
    """[D, C] -> [128, D//128, C] partition-tiled so the device DMA is
    contiguous: out[p, t, c] = w[t*128 + p, c]."""
    Dd, C = w.shape
    return np.ascontiguousarray(
        w.reshape(Dd // 128, 128, C).transpose(1, 0, 2)
    )


def make_in_maps(x, Wq, Wk, Wv, Wo, n_cores=8):
    """x: (T, D) fp32. Returns per-core input maps (bf16 operands)."""
    T = x.shape[0]
    ich = min(ICH, T)
    # [nch, 128, NT, ich]: chunk-contiguous so each chunk is one linear DMA
    xT = np.ascontiguousarray(
        x.T.astype(NPBF)
        .reshape(NT, 128, T // ich, ich)
        .transpose(2, 1, 0, 3)
    )
    maps = []
    for n in range(n_cores):
        sl = slice(CPC * n, CPC * (n + 1))
        maps.append(
            {
                "xT": xT,
                "wqT": _tile_dk(Wq[sl, :].T.astype(NPBF)),
                "wkT": _tile_dk(Wk[sl, :].T.astype(NPBF)),
                "wvT": _tile_dk(Wv[sl, :].T.astype(NPBF)),
                "woT": np.ascontiguousarray(Wo[:, sl].T).astype(NPBF),
                "tri": TRI,
                "ident": IDENT,
            }
        )
    return maps


def run(x, Wq, Wk, Wv, Wo, T=None, n_cores=8, trace=False):
    global LAST_RESULTS
    T = T if T is not None else x.shape[0]
    nc = get_nc(T)
    in_maps = make_in_maps(x, Wq, Wk, Wv, Wo, n_cores)
    res = run_bass_kernel_spmd(
        nc, in_maps, core_ids=list(range(n_cores)), trace=trace
    )
    LAST_RESULTS = res
    y = np.zeros((T, D), dtype=np.float64)
    for r in res.results:
        y += r["y"].astype(np.float64)
    return y.astype(np.float32)


def kernel(x, Wq, Wk, Wv, Wo):
    x = np.asarray(x, dtype=np.float32)
    B, T, _ = x.shape
    trace = bool(os.environ.get("MHA_TRACE"))
    y = run(
        np.ascontiguousarray(x.reshape(T, D)),
        np.asarray(Wq, np.float32),
        np.asarray(Wk, np.float32),
        np.asarray(Wv, np.float32),
        np.asarray(Wo, np.float32),
        T=T,
        trace=trace,
    )
    if trace and LAST_RESULTS is not None and LAST_RESULTS.exec_time_ns:
        print(f"HW exec time: {LAST_RESULTS.exec_time_ns} ns")
    return y.reshape(B, T, D)


# revision 62
# speedup vs baseline: 1.0483x; 1.0061x over previous
# Multi-head causal attention (B=1, T=4096, D=1024, H=16) on 8 TRN2 NeuronCores.
#
# Sharding: tensor-parallel over heads. Core n computes head channels
# [128n, 128n+128) (= heads 2n, 2n+1), runs the full causal attention for its
# two heads, and produces a full-shape partial output
#   y_n = attn_out[:, ch_n] @ Wo[:, ch_n].T        (4096, 1024)
# The host sums the 8 partials (row-sharded Wo contraction) — no collectives.
#
# Device-side layout (per core):
#   xT   [128, 8, T]   x transposed + partition-tiled on the host so the DMA
#                      is contiguous; contraction (d_model) on partitions.
#   QT/KT [128, 2, T]  head channels on partitions (h0: 0-63, h1: 64-127), bf16
#   scoresT[j, i]      keys on partitions, queries on free dim; the softmax sum
#                      over keys rides the PV matmul via a ones-column appended
#                      to V (V' = [V | 1], M=65): psum row 64 = rowsum.
#   exp on ScalarE directly PSUM->SBUF (bf16 out) with 1/sqrt(dk) folded into
#   the activation scale; psum accumulation is always fp32.
#   Causal: only key tiles with j <= i are computed; diagonal 128x128 blocks
#   are masked by a 0/1 upper-triangular multiply after exp.
#
# Schedule: the ScalarE exp stream (144 ACTIVATEs of 128x1024, ~1.15us each,
# (N+352)/1.2ns) is the critical resource; everything is arranged to keep it
# saturated. Attention is a flat sequence of per-128-key-tile steps per
# 512-query chunk; both heads' K=64 score matmuls are row-tiled
# (tile_position auto-derives from the partition bases: h0 rows 0-63, h1
# rows 64-127) and execute CONCURRENTLY in the PE array — one N=512 window
# per key tile. At step k the kernel emits score matmuls for step k+1, the
# exp for step k, and PV matmuls for step k-2 (the 2-step lag keeps
# psum-handoff waits off the in-order PE queue's critical path; the last two
# PV pairs are emitted after the NEXT chunk's first score window).
# PSUM banks: 2 rotating 2-bank score tiles (4), one 2-bank aux slot for
# projection/normalization/output tiles (2), two 1-bank PV accumulators (2).
# The Q/K/V projection of chunk c+1 and the tail of chunk c-1 interleave
# into chunk c's attention steps via gated generators: softmax
# normalization broadcasts the rowsums with two K=1 ones-matmuls and
# inverts them with the fast approx DVE reciprocal (no DRAM roundtrip);
# the y matmuls are gated late so they never head-of-line block the PE,
# and early (PE-oversubscribed) chunks defer them two chunks.
# Prologue: exp-table + HAM-clock warm-up overlap the ordered weight/x DMAs.

import os
import sys

for _p in ("/opt/trn_rl_repo", "/root/.axon_site/_ro/trn_rl_repo"):
    if os.path.isdir(_p) and _p not in sys.path:
        sys.path.insert(0, _p)

import ml_dtypes
import numpy as np

def _ensure_axon_ntff_hook():
    """The agent image's antenv package lacks axon_hooks, which makes
    run_bass_kernel_spmd(trace=True) crash at import under axon. Provide the
    module and register the boot hook so NTFF profiling works."""
    import types

    try:
        import antenv.axon_hooks  # noqa: F401
        return
    except ImportError:
        pass
    try:
        import antenv
    except ImportError:
        return
    mod = types.ModuleType("antenv.axon_hooks")
    mod._hook = None
    mod.set_axon_ntff_profile_hook = lambda h: setattr(mod, "_hook", h)
    mod.get_axon_ntff_profile_hook = lambda: mod._hook
    sys.modules["antenv.axon_hooks"] = mod
    antenv.axon_hooks = mod
    try:
        from trn_agent_boot.trn_boot import _ntff_profile_via_ctypes

        so = "/opt/axon/libaxon_pjrt.so"
        if os.path.exists(so):
            mod._hook = _ntff_profile_via_ctypes(so)
    except Exception:
        pass


_ensure_axon_ntff_hook()

import concourse.bass as bass
import concourse.tile as tile
from concourse import bacc
from concourse import mybir
from concourse.bass_utils import run_bass_kernel_spmd

F32 = mybir.dt.float32
BF16 = mybir.dt.bfloat16
EXP = mybir.ActivationFunctionType.Exp
NPBF = ml_dtypes.bfloat16

D = 1024          # d_model
DK = 64           # head dim
CPC = 128         # channels per core (2 heads)
ICH = 512         # query-chunk size
IH = 512          # i-half width (matmul N / psum bank limit)
JT = 128          # key-tile size
NT = D // 128     # d_model tiles

_NC_CACHE = {}


def build(T):
    """Build the per-core Bass program for sequence length T."""
    nc = bacc.Bacc(None, target_bir_lowering=False, debug=False)
    ich = min(ICH, T)
    nch = T // ich

    xT_d = nc.dram_tensor(
        "xT", [T // ICH if T >= ICH else 1, 128, NT, min(ICH, T)], BF16,
        kind="ExternalInput",
    )
    wqT_d = nc.dram_tensor("wqT", [128, NT, CPC], BF16, kind="ExternalInput")
    wkT_d = nc.dram_tensor("wkT", [128, NT, CPC], BF16, kind="ExternalInput")
    wvT_d = nc.dram_tensor("wvT", [128, NT, CPC], BF16, kind="ExternalInput")
    woT_d = nc.dram_tensor("woT", [CPC, D], BF16, kind="ExternalInput")
    tri_d = nc.dram_tensor("tri", [JT, JT], BF16, kind="ExternalInput")
    ident_d = nc.dram_tensor("ident", [128, 128], BF16, kind="ExternalInput")
    y_d = nc.dram_tensor("y", [T, D], BF16, kind="ExternalOutput")

    with tile.TileContext(nc) as tc:
        with (
            tc.tile_pool(name="const", bufs=1) as const,
            tc.tile_pool(name="xtp", bufs=2) as xtp,
            tc.tile_pool(name="vtp", bufs=2) as vtp,
            tc.tile_pool(name="expp", bufs=6) as expp,
            tc.tile_pool(name="outp", bufs=2) as outp,
            tc.tile_pool(name="yp", bufs=4) as yp,
            tc.tile_pool(name="psp", bufs=2, space="PSUM") as psp,
            tc.tile_pool(name="pvp", bufs=1, space="PSUM") as pvp,
        ):
            xt_tiles = {}

            def prefetch_xt(c):
                xt_ch = xtp.tile([128, NT, ich], BF16, tag="xt", name="xt_ch")
                nc.sync.dma_start(out=xt_ch, in_=xT_d[c])
                xt_tiles[c] = xt_ch

            # ---- warm the ScalarE exp table while the first DMAs run ----
            warm = const.tile([128, 1], F32)
            nc.vector.memset(warm, 0.0)
            nc.scalar.activation(out=warm, in_=warm, func=EXP)

            # ---- HAM warm-up: ~3.5us of dummy matmuls (on garbage SBUF, to
            # a throwaway psum bank) so the PE clock-gate opens to 2.4 GHz
            # before the real projection matmuls arrive ----
            junk = const.tile([128, 128], BF16)
            nc.vector.memset(junk, 1.0)
            warm_ps = psp.tile([128, 2, IH], F32, tag="aux", bufs=1, name="warm_ps")
            for _ in range(30):
                nc.tensor.matmul(
                    out=warm_ps[:, 0, 0:128], lhsT=junk, rhs=junk,
                    start=True, stop=True,
                )

            # ---- constants / persistent state; DMA trigger order matters:
            # the sync queue issues one ~0.6us trigger at a time, so the
            # operands of the first projection matmuls go first ----
            wq_sb = const.tile([128, NT, 128], BF16)
            wk_sb = const.tile([128, NT, 128], BF16)
            wv_sb = const.tile([128, NT, 128], BF16)
            wo_sb = const.tile([128, D], BF16)
            tri_sb = const.tile([JT, JT], BF16)
            id_sb = const.tile([128, 128], BF16)

            xt_ch0 = xtp.tile([128, NT, ich], BF16, tag="xt", name="xt_ch")
            xt_tiles[0] = xt_ch0
            nc.sync.dma_start(out=wq_sb, in_=wqT_d[:, :, :])
            nc.sync.dma_start(out=xt_ch0[:, 0:2, :], in_=xT_d[0, :, 0:2, :])
            nc.sync.dma_start(out=xt_ch0[:, 2:NT, :], in_=xT_d[0, :, 2:NT, :])
            nc.sync.dma_start(out=wk_sb, in_=wkT_d[:, :, :])
            nc.sync.dma_start(out=wv_sb, in_=wvT_d[:, :, :])
            if nch > 1:
                prefetch_xt(1)
            nc.sync.dma_start(out=wo_sb, in_=woT_d[:, :])
            nc.sync.dma_start(out=tri_sb, in_=tri_d[:, :])
            nc.sync.dma_start(out=id_sb, in_=ident_d[:, :])

            ones_col = const.tile([1, DK], F32)
            nc.vector.memset(ones_col, 1.0)

            qt_sb = const.tile([128, 2, T], BF16)  # [:,0,:]=QT, [:,1,:]=KT
            # V' = [V_h | 1] per head: [j, jt, 2*65]
            vp_sb = const.tile([128, T // JT, 2 * (DK + 1)], BF16)
            ones_view = vp_sb.rearrange("p t (h c) -> p t h c", h=2)[
                :, :, :, DK : DK + 1
            ]
            nc.vector.memset(ones_view, 1.0)

            def gen_proj(c):
                """Generator emitting the Q/K/V projection for chunk c,
                one PE/DVE op per yield (the x chunk was prefetched).
                NOTE: must be fully emitted before chunk c's attention —
                the vp_sb copy's rearranged write is not reliably ordered
                against the PV matmuls by the dependency tracker."""
                i0 = c * ich
                xt_ch = xt_tiles.pop(c)
                # interleave the Q and K accumulation chains: consecutive
                # matmuls into the SAME psum bank pace at ~429ns (the bank
                # read-modify-write can't overlap the next fill) while
                # bank-alternating matmuls pace at ~215ns
                qk_ps = psp.tile([128, 2, ich], F32, tag="aux", bufs=1, name="qk_ps")
                for qk, w_sb in ((0, wq_sb), (1, wk_sb)):
                    for t in range(NT):
                        nc.tensor.matmul(
                            out=qk_ps[:, qk, :],
                            lhsT=w_sb[:, t, :],
                            rhs=xt_ch[:, t, :],
                            start=(t == 0),
                            stop=(t == NT - 1),
                        )
                        yield
                nc.vector.tensor_copy(out=qt_sb[:, :, i0 : i0 + ich], in_=qk_ps)
                yield
                vt_ps = psp.tile([128, 2, ich], F32, tag="aux", bufs=1, name="vt_ps")
                for t in range(NT):
                    nc.tensor.matmul(
                        out=vt_ps[:, 0, :],
                        lhsT=wv_sb[:, t, :],
                        rhs=xt_ch[:, t, :],
                        start=(t == 0),
                        stop=(t == NT - 1),
                    )
                    yield
                vt_sb = vtp.tile([128, ich], BF16, tag="vt", name="vt_sb")
                nc.vector.tensor_copy(out=vt_sb, in_=vt_ps[:, 0, :])
                yield
                vn_ps = psp.tile(
                    [128, ich // 128, 128], BF16, tag="aux", bufs=1, name="vn_ps"
                )
                for sdx in range(ich // 128):
                    nc.tensor.transpose(
                        out=vn_ps[:, sdx, :],
                        in_=vt_sb[:, sdx * 128 : (sdx + 1) * 128],
                        identity=id_sb,
                    )
                    yield
                jt0 = i0 // JT
                nc.vector.tensor_copy(
                    out=vp_sb.rearrange("p t (h c) -> p t h c", h=2)[
                        :, jt0 : jt0 + ich // 128, :, 0:DK
                    ],
                    in_=vn_ps.rearrange("p s (h c) -> p s h c", h=2),
                )
                yield

            def gen_tail_head(c, pv, out_slot, last=False):
                """Rowsum + raw-output extraction out of the pv psum banks —
                emitted first thing in the next chunk so the banks free up
                for its PV accumulation. After the final chunk the scalar
                engine is idle, so the rowsum copies run there, in parallel
                with the vector-queue outt copies."""
                rs_sb = outp.tile([1, 2, ich], F32, tag="rs", name="rs_sb")
                for h in range(2):
                    if last:
                        nc.scalar.copy(
                            out=rs_sb[0:1, h, :], in_=pv[h][DK : DK + 1, :]
                        )
                    else:
                        nc.vector.tensor_copy(
                            out=rs_sb[0:1, h, :], in_=pv[h][DK : DK + 1, :]
                        )
                outt = outp.tile([128, ich], BF16, tag="outt", name="outt")
                for h in range(2):
                    nc.vector.tensor_copy(
                        out=outt[h * DK : (h + 1) * DK, :], in_=pv[h][0:DK, :]
                    )
                out_slot["rs"] = rs_sb
                out_slot["outt"] = outt
                yield
                # rowsum broadcast to the 64 head channels via two K=1
                # ones-matmuls (cheaper and far lower latency than a
                # scratch-DRAM roundtrip); needs only rs_sb, so the PE
                # reaches it with its input ready
                bc_ps = psp.tile(
                    [128, 2, IH], F32, tag="aux", bufs=1, name="bc_ps"
                )
                for h in range(2):
                    nc.tensor.matmul(
                        out=bc_ps[0:DK, h, :],
                        lhsT=ones_col,
                        rhs=rs_sb[0:1, h, :],
                        start=True,
                        stop=True,
                    )
                out_slot["bc_ps"] = bc_ps
                yield

            def gen_tail_bc(c, out_slot):
                """DVE-only normalization: fast approximate reciprocal
                (18 bits — plenty for a softmax denominator; rowsums are
                strictly positive) of the broadcast rowsums, then scale the
                raw attention output. No PE ops — cannot stall the PE
                queue."""
                outt, bc_ps = out_slot["outt"], out_slot["bc_ps"]
                # the custom-DVE approx reciprocal requires matching
                # partition bases, so invert at base 0 then shift-copy
                bct = outp.tile([DK, 2, ich], F32, tag="bct", name="bct")
                for h in range(2):
                    nc.vector.reciprocal_approx_fast(
                        out=bct[:, h, :], in_=bc_ps[0:DK, h, :]
                    )
                yield
                bc = outp.tile([128, ich], F32, tag="bc", name="bc")
                for h in range(2):
                    nc.vector.tensor_copy(
                        out=bc[h * DK : (h + 1) * DK, :], in_=bct[:, h, :]
                    )
                    yield
                # per-half muls so the first y matmuls (reading the first
                # 256 queries) can start before the whole tile is scaled
                for q in (slice(0, ich // 2), slice(ich // 2, ich)):
                    nc.vector.tensor_mul(
                        outt[:, q], outt[:, q], bc[:, q]
                    )
                    yield

            def gen_tail_y(c, out_slot, last=False):
                """Output projection, gated late so the y matmuls reach the
                in-order PE queue only after the normalization chain is
                done. The last chunk's y tiles use the score rotation (free
                by then) so its output projection pipelines 2-deep."""
                i0 = c * ich
                outt = out_slot["outt"]
                for sidx in range(ich // 128):
                    y_ps = psp.tile(
                        [128, 2, IH], F32,
                        tag="sc" if last else "aux",
                        bufs=2 if last else 1,
                        name="y_ps",
                    )
                    for e in range(2):
                        nc.tensor.matmul(
                            out=y_ps[:, e, :],
                            lhsT=outt[:, sidx * 128 : (sidx + 1) * 128],
                            rhs=wo_sb[:, e * IH : (e + 1) * IH],
                            start=True,
                            stop=True,
                        )
                        yield
                    y_sb = yp.tile([128, D], BF16, tag="y", name="y_sb")
                    nc.vector.tensor_copy(
                        out=y_sb, in_=y_ps.rearrange("p a b -> p (a b)")
                    )
                    yield
                    r0 = i0 + sidx * 128
                    nc.sync.dma_start(out=y_d[r0 : r0 + 128, :], in_=y_sb)
                    yield

            def emit_chunk(c, pending):
                """Attention steps for chunk c, draining `pending` generator
                entries [min_frac, gen] into the step slack."""
                i0 = c * ich
                njt = (i0 + ich) // JT
                pv = [
                    pvp.tile([128, ich], F32, tag="pv0", name="pv0"),
                    pvp.tile([128, ich], F32, tag="pv1", name="pv1"),
                ]
                # one step per 128-key tile; both heads' K=64 score matmuls
                # are row-tiled (tile_position auto-derives from the lhsT/out
                # base partitions: h0 rows 0-63, h1 rows 64-127) and execute
                # CONCURRENTLY in the PE array — one N=512 window for both
                nsteps = njt
                sc_tiles = {}
                ex_tiles = {}

                def emit_sc(k):
                    sc = psp.tile([128, 2, IH], F32, tag="sc", name="sc")
                    sc_tiles[k] = sc
                    for h in range(2):
                        hp = slice(h * DK, (h + 1) * DK)
                        nc.tensor.matmul(
                            out=sc[:, h, :],
                            lhsT=qt_sb[hp, 1, k * JT : (k + 1) * JT],
                            rhs=qt_sb[hp, 0, i0 : i0 + ich],
                            start=True,
                            stop=True,
                        )

                def emit_exp(k):
                    sc = sc_tiles.pop(k)
                    ex = expp.tile([128, 2, IH], BF16, tag="ex", name="ex")
                    ex_tiles[k] = ex
                    nc.scalar.activation(
                        out=ex, in_=sc, func=EXP, scale=1.0 / np.sqrt(DK)
                    )
                    off = k * JT - i0
                    if off >= 0:  # diagonal tile: causal mask
                        if off > 0:
                            nc.vector.memset(ex[:, :, 0:off], 0.0)
                        for h in range(2):
                            nc.vector.tensor_mul(
                                ex[:, h, off : off + JT],
                                ex[:, h, off : off + JT],
                                tri_sb,
                            )

                def emit_pv(k):
                    ex = ex_tiles.pop(k)
                    for h in range(2):
                        nc.tensor.matmul(
                            out=pv[h][0 : DK + 1, :],
                            lhsT=vp_sb[:, k, h * (DK + 1) : (h + 1) * (DK + 1)],
                            rhs=ex[:, h, :],
                            start=(k == 0),
                            stop=(k == njt - 1),
                        )

                def drain(budget, frac):
                    budget += 2 if frac > 0.8 else 0
                    while budget > 0:
                        eligible = [
                            e for e in pending
                            if e[2] < c or (e[2] == c and e[0] <= frac)
                        ]
                        if not eligible:
                            return
                        progressed = False
                        for e in eligible:
                            if budget <= 0:
                                return
                            try:
                                next(e[1])
                                budget -= 1
                                progressed = True
                            except StopIteration:
                                pending.remove(e)
                        if not progressed:
                            return

                total_ops = sum(
                    20 if e[0] == 0.0 else 14
                    for e in pending if e[2] <= c
                )
                emit_sc(0)
                # the previous chunk's 2 lagging PV pairs are emitted here,
                # AFTER this chunk's first score window, so the next exp is
                # never queued behind them at the boundary
                for fn in prev_trailing:
                    fn()
                for k in range(nsteps):
                    if k + 1 < nsteps:
                        emit_sc(k + 1)
                    emit_exp(k)
                    if k - 2 >= 0:
                        emit_pv(k - 2)
                    drain(
                        (total_ops + nsteps - 1 - k) // nsteps + 1,
                        (k + 1) / nsteps,
                    )
                trailing = [
                    (lambda k=k: emit_pv(k))
                    for k in (nsteps - 2, nsteps - 1)
                ]
                return pv, trailing

            # ---- main schedule ----
            for _ in gen_proj(0):
                pass
            pending = []
            prev_trailing = []
            for c in range(nch):
                if c + 2 < nch:
                    prefetch_xt(c + 2)
                if c + 1 < nch:
                    pending.append([0.0, gen_proj(c + 1), c])
                pv, prev_trailing = emit_chunk(c, pending)
                out_slot = {}
                pending.append(
                    [0.0, gen_tail_head(c, pv, out_slot, last=(c == nch - 1)),
                     c + 1]
                )
                pending.append([0.20, gen_tail_bc(c, out_slot), c + 1])
                # early chunks are PE-oversubscribed (attention + next
                # chunk's projection exceed their few steps of scalar
                # time), so their output projections run two chunks later
                # where the bigger chunks have PE slack
                ydef = c + 2 if c < nch - 3 else c + 1
                pending.append(
                    [0.55, gen_tail_y(c, out_slot, last=(c == nch - 1)), ydef]
                )
            for fn in prev_trailing:
                fn()
            for e in pending:
                for _ in e[1]:
                    pass
    nc.compile()
    return nc


def get_nc(T):
    if T not in _NC_CACHE:
        _NC_CACHE[T] = build(T)
    return _NC_CACHE[T]


TRI = np.triu(np.ones((JT, JT))).astype(NPBF)  # 1 where key j <= query i
IDENT = np.eye(128).astype(NPBF)

LAST_RESULTS = None  # BassKernelResults of the last run (for profiling)


def _tile_dk(w):
    """[D, C] -> [128, D//128, C] partition-tiled so the device DMA is
    contiguous: out[p, t, c] = w[t*128 + p, c]."""
    Dd, C = w.shape
    return np.ascontiguousarray(
        w.reshape(Dd // 128, 128, C).transpose(1, 0, 2)
    )


def make_in_maps(x, Wq, Wk, Wv, Wo, n_cores=8):
    """x: (T, D) fp32. Returns per-core input maps (bf16 operands)."""
    T = x.shape[0]
    ich = min(ICH, T)
    # [nch, 128, NT, ich]: chunk-contiguous so each chunk is one linear DMA
    xT = np.ascontiguousarray(
        x.T.astype(NPBF)
        .reshape(NT, 128, T // ich, ich)
        .transpose(2, 1, 0, 3)
    )
    maps = []
    for n in range(n_cores):
        sl = slice(CPC * n, CPC * (n + 1))
        maps.append(
            {
                "xT": xT,
                "wqT": _tile_dk(Wq[sl, :].T.astype(NPBF)),
                "wkT": _tile_dk(Wk[sl, :].T.astype(NPBF)),
                "wvT": _tile_dk(Wv[sl, :].T.astype(NPBF)),
                "woT": np.ascontiguousarray(Wo[:, sl].T).astype(NPBF),
                "tri": TRI,
                "ident": IDENT,
            }
        )
    return maps


def run(x, Wq, Wk, Wv, Wo, T=None, n_cores=8, trace=False):
    global LAST_RESULTS
    T = T if T is not None else x.shape[0]
    nc = get_nc(T)
    in_maps = make_in_maps(x, Wq, Wk, Wv, Wo, n_cores)
    res = run_bass_kernel_spmd(
        nc, in_maps, core_ids=list(range(n_cores)), trace=trace
    )
    LAST_RESULTS = res
    y = np.zeros((T, D), dtype=np.float64)
    for r in res.results:
        y += r["y"].astype(np.float64)
    return y.astype(np.float32)


def kernel(x, Wq, Wk, Wv, Wo):
    x = np.asarray(x, dtype=np.float32)
    B, T, _ = x.shape
    trace = bool(os.environ.get("MHA_TRACE"))
    y = run(
        np.ascontiguousarray(x.reshape(T, D)),
        np.asarray(Wq, np.float32),
        np.asarray(Wk, np.float32),
        np.asarray(Wv, np.float32),
        np.asarray(Wo, np.float32),
        T=T,
        trace=trace,
    )
    if trace and LAST_RESULTS is not None and LAST_RESULTS.exec_time_ns:
        print(f"HW exec time: {LAST_RESULTS.exec_time_ns} ns")
    return y.reshape(B, T, D)
